# revision 46
# baseline (speedup 1.0000x reference)
"""Trainium2 Bass kernel for causal linear-complexity multi-head attention.

Reference computation (per batch n):
    q = softmax((query @ Wq.T) per-head, axis=Dh)
    k = softmax((key  @ Wk.T) per-head, axis=Dh)
    v = key @ Wv.T
    S[t] = sum_{s<=t} k_s^T v_s          (per-head Dh x Dh running state)
    out[t] = q_t @ S[t]

Sharding: 8 cores = 2 batches x 4 head-groups (4 heads of 64 dims each per
core).  Each core gets host-transposed inputs (d on rows); per-core output
is the natural-layout (L x 256) slice; the host concatenates.

Device algorithm: chunked linear attention, chunk C=256.

Projections run as fp8e4m3 DoubleRow matmuls (2 contraction planes per
matmul, 0.5 cycles/row).  W ships as an fp8 hi/lo split (residual
re-rounded); x ships as fp8 hi (plus a lo residual plane for the early
v chunks).  The per-projection term count trades error for passes:
    q:         Wh@x8 + Wl@x8                       (2 passes, all chunks)
    k:         + Wl term only for chunks < K_LO_CHUNKS (else 1 pass)
    v:         + Wh@xlo term only for chunks < V_LO_CHUNKS
Early chunks keep more terms because their k/v errors persist in the
cumulative state S and touch every later output; late-chunk errors only
touch the few outputs after them.  Measured rel err 1.76e-2 vs the 2e-2
gate.  W is pre-scaled by 64 on host to center it in fp8 normal range;
undone by the exp activation scale (q/k paths) and a 64-valued
k-denominator ones vector (v path, scale rides the per-head reciprocal).

The attention block runs in bf16 and produces natural-layout output:
    scores   pat[s, t] = ek^T eq        (per head, T layout)
    output   out[t, j] = at^T v + eq^T S
Causal-structure zero blocks are never computed or touched:
  - score tile st=1 stores its diagonal block at columns 0:128, so both
    128x128 triangular blocks of a (head, t-tile) sit at a single strided
    AP and one DVE multiply masks both; the st=0 "all ones" block is a
    plain ACT copy (no mask needed).
  - the t-tile-0 output matmul skips the st=1 (all zero) score block.
The running state S accumulates IN PSUM across chunks (start only on the
very first update; later updates ride the accumulation bit), with a bf16
diagonal shadow extracted on ACT for the apply matmuls — no vector-engine
adds on the state path.  A PE warmup loop of identity transposes burns
the tensor-engine p-state ramp while the first input DMAs land; the
startup DMA stream is ordered by first use (the HWDGE descriptor
generator and the DMA wire are both serial devices).
"""

import threading
from contextlib import ExitStack

import numpy as np

import concourse.bass as bass
import concourse.mybir as mybir
import concourse.tile as tile
from concourse import bacc
from concourse.bass_utils import run_bass_kernel_spmd

P = 128          # SBUF partitions
D = 1024         # model dim (contraction)
J = 256          # per-core output columns (4 heads x 64)
L = 2048         # sequence length
C = 256          # chunk size
NCH = L // C     # chunks
DH = 64          # per-head dim
KB = 4           # 256-deep contraction blocks
N_CORES = 8
WSCALE = 64.0    # host pre-scale on W for fp8 range

# error/speed dials: how many leading chunks keep the extra fp8
# compensation terms (early-chunk k/v errors persist in the state S)
Q_LO = False      # q never ships an x residual
K_LO_CHUNKS = 3   # chunks with the 2-term (W-residual) k projection
V_LO = True
V_LO_CHUNKS = 5   # chunks with the 3-term v projection
WARMUP = 16       # identity transposes to burn the PE p-state ramp

F32 = mybir.dt.float32
BF16 = mybir.dt.bfloat16
FP8 = mybir.dt.float8e4
EXP = mybir.ActivationFunctionType.Exp
COPY = mybir.ActivationFunctionType.Copy
DR = mybir.MatmulPerfMode.DoubleRow

TERMS2 = ((0, 0), (1, 0))              # (w hi/lo, x plane)
TERMS1 = ((0, 0),)
TERMS2X = ((0, 0), (0, 1))             # W hi with x hi + x lo (k path)
TERMS3 = ((0, 0), (1, 0), (0, 1))      # + W_hi @ x_lo correction
XQH = 2 if Q_LO else 1
XKH = 2 if V_LO else 1


def _build_nc():
    nc = bacc.Bacc(trn_type="TRN2", target_bir_lowering=False, num_devices=N_CORES)

    # x: [p, chunk, hi(/lo), kb, plane, t]  with d = kb*256 + plane*128 + p
    xq = nc.dram_tensor("xq", [P, NCH, XQH, KB, 2, C], FP8, kind="ExternalInput").ap()
    xk = nc.dram_tensor("xk", [P, NCH, XKH, KB, 2, C], FP8, kind="ExternalInput").ap()
    # w: [p, proj(q,k,v), hi/lo, kb, plane, j]
    wa = nc.dram_tensor("wa", [P, 3, 2, KB, 2, J], FP8, kind="ExternalInput").ap()
    # packed consts: [0:128] ident, [128:256] tri mask, [256:258] ones,
    # [258:260] 64*ones
    cst = nc.dram_tensor("cst", [P, 260], BF16, kind="ExternalInput").ap()
    out_d = nc.dram_tensor("out_nat", [L, J], BF16, kind="ExternalOutput").ap()

    # natural-layout store: row = c*256 + tt*128 + p
    out_r = out_d.rearrange("(c tt p) j -> p c tt j", p=P, tt=2)

    with tile.TileContext(nc) as tc, ExitStack() as ctx:
        ctx.enter_context(
            nc.allow_low_precision(reason="fp8/bf16 compensated pipeline")
        )
        cpool = ctx.enter_context(tc.tile_pool(name="consts", bufs=1))
        xpool = ctx.enter_context(tc.tile_pool(name="xin", bufs=6))
        spool = ctx.enter_context(tc.tile_pool(name="sb", bufs=4))
        apool = ctx.enter_context(tc.tile_pool(name="at", bufs=8))
        ppool = ctx.enter_context(tc.tile_pool(name="pp", bufs=2, space="PSUM"))
        patp = ctx.enter_context(tc.tile_pool(name="pa", bufs=3, space="PSUM"))
        potp = ctx.enter_context(tc.tile_pool(name="po", bufs=1, space="PSUM"))
        pstat = ctx.enter_context(tc.tile_pool(name="ps", bufs=1, space="PSUM"))
        pnorm = ctx.enter_context(tc.tile_pool(name="pn", bufs=1, space="PSUM"))

        # ---- startup DMA stream (HWDGE is serial: needed-first order) ----
        cst_sb = cpool.tile([P, 260], BF16, tag="cst_sb")
        nc.sync.dma_start(cst_sb[:], cst[:])
        ident_sb = cst_sb[:, 0:P]
        tri_sb = cst_sb[:, P:2 * P]
        eones1_sb = cst_sb[:, 2 * P:2 * P + 2]
        eones64_sb = cst_sb[:, 2 * P + 2:2 * P + 4]

        # PE p-state warmup: transposes on the (tiny, first-landing) ident
        warm = pnorm.tile([P, P], BF16, tag="pn", name="warm")
        for _ in range(WARMUP):
            nc.tensor.transpose(warm[:], ident_sb, ident_sb)

        wa_sb = cpool.tile([P, 3, 2, KB, 2, J], FP8, tag="wa_sb")
        xq_t0 = xpool.tile([P, XQH, KB, 2, C], FP8, tag="xq_t", name="xq_t0")
        xk_t0 = xpool.tile([P, XKH, KB, 2, C], FP8, tag="xk_t", name="xk_t0")
        nc.sync.dma_start(wa_sb[:, 0, 0, 0:2], wa[:, 0, 0, 0:2])
        nc.sync.dma_start(xq_t0[:, 0, 0:2], xq[:, 0, 0, 0:2])
        nc.sync.dma_start(wa_sb[:, 0, 1], wa[:, 0, 1])
        nc.sync.dma_start(wa_sb[:, 0, 0, 2:4], wa[:, 0, 0, 2:4])
        nc.sync.dma_start(xq_t0[:, 0, 2:4], xq[:, 0, 0, 2:4])
        if Q_LO:
            nc.sync.dma_start(xq_t0[:, 1], xq[:, 0, 1])
        nc.sync.dma_start(wa_sb[:, 1, 0], wa[:, 1, 0])
        nc.sync.dma_start(xk_t0[:, 0], xk[:, 0, 0])
        nc.sync.dma_start(wa_sb[:, 1, 1], wa[:, 1, 1])
        if V_LO:
            nc.sync.dma_start(xk_t0[:, 1], xk[:, 0, 1])
        nc.sync.dma_start(wa_sb[:, 2, 0], wa[:, 2, 0])
        nc.sync.dma_start(wa_sb[:, 2, 1], wa[:, 2, 1])
        xq_t1 = xpool.tile([P, XQH, KB, 2, C], FP8, tag="xq_t", name="xq_t1")
        nc.sync.dma_start(xq_t1[:], xq[:, 1])
        xk_t1 = xpool.tile([P, XKH, KB, 2, C], FP8, tag="xk_t", name="xk_t1")
        nc.sync.dma_start(xk_t1[:, 0], xk[:, 1, 0])
        if V_LO:
            nc.sync.dma_start(xk_t1[:, 1], xk[:, 1, 1])

        S16 = cpool.tile([P, 2, DH], BF16, tag="S16")
        # resident PSUM accumulator for the running state (diag head blocks
        # of ekn^T v land at [64h:64h+64, jt, 64h:64h+64])
        pds = pstat.tile([P, 2, P], F32, tag="ps")

        qterms = TERMS3 if Q_LO else TERMS2

        def vterms(c):
            # early chunks feed every later output through S: keep the x
            # residual there; late chunks touch few outputs, drop it
            return (TERMS3 if c < V_LO_CHUNKS else TERMS2) if V_LO else TERMS2

        def dr_proj_T(pe_t, proj, x_t, terms):
            """q/k projection in transposed layout: out[j, t] per jt."""
            for jt in range(2):
                n = 0
                for whl, xhl in terms:
                    for kb in range(KB):
                        nc.tensor.matmul(
                            pe_t[:, jt, :],
                            wa_sb[:, proj, whl, kb, :, jt * P:(jt + 1) * P],
                            x_t[:, xhl, kb, :, :],
                            start=(n == 0),
                            stop=(n == len(terms) * KB - 1),
                            perf_mode=DR,
                        )
                        n += 1

        def state_update(c, ekn_sb, v_sb):
            # S += ekn^T v for chunk c, accumulated in the resident PSUM
            # bank (start only opens the zero region once, on the very
            # first matmul; later chunks accumulate in place)
            for jt in range(2):
                for st in range(2):
                    nc.tensor.matmul(
                        pds[:, jt, :],
                        ekn_sb[:, st, jt * P:(jt + 1) * P],
                        v_sb[:, st, jt * P:(jt + 1) * P],
                        start=(c == 0 and jt == 0 and st == 0),
                        stop=(st == 1),
                        skip_group_check=True,
                    )
            # bf16 diag shadow for the next chunk's apply matmuls; ACT is
            # idle at chunk end and the copy has a full chunk of slack
            for half in range(2):
                rows = slice(64 * half, 64 * half + 64)
                cols = slice(64 * half, 64 * half + 64)
                nc.scalar.activation(S16[rows, :, :], pds[rows, :, cols], COPY)

        xtiles = {0: (xq_t0, xk_t0), 1: (xq_t1, xk_t1)}
        for c in range(NCH):
            last = c == NCH - 1

            if c + 2 < NCH:
                xq_n = xpool.tile([P, XQH, KB, 2, C], FP8, tag="xq_t", name="xq_n")
                nc.sync.dma_start(xq_n[:], xq[:, c + 2])
                xk_n = xpool.tile([P, XKH, KB, 2, C], FP8, tag="xk_t", name="xk_n")
                if len(vterms(c + 2)) == 3:
                    nc.sync.dma_start(xk_n[:], xk[:, c + 2])
                else:
                    nc.sync.dma_start(xk_n[:, 0], xk[:, c + 2, 0])

                xtiles[c + 2] = (xq_n, xk_n)
            xq_t, xk_t = xtiles.pop(c)

            # ---- q/k projections (transposed layout) + exp ----
            eq_e = spool.tile([P, 2, C], BF16, tag="eq_e")
            ek_e = spool.tile([P, 2, C], BF16, tag="ek_e")
            pq_t = ppool.tile([P, 2, C], F32, tag="pp")
            dr_proj_T(pq_t, 0, xq_t, qterms)
            # high priority: the exps jump the previous chunk's queued ACT
            # tail (ones/ekn/S16) the moment the projection lands, instead
            # of draining behind it -- removes the pdq/scores eq-wait
            with tc.high_priority():
                nc.scalar.activation(eq_e[:], pq_t[:], EXP, scale=1.0 / WSCALE)
            pk_t = ppool.tile([P, 2, C], F32, tag="pp")
            dr_proj_T(pk_t, 1, xk_t, TERMS2 if c < K_LO_CHUNKS else TERMS1)
            with tc.high_priority():
                nc.scalar.activation(ek_e[:], pk_t[:], EXP, scale=1.0 / WSCALE)

            # ---- q denominators (natural layout, tiny) ----
            # pdq[t, tt, jt, h2] = dq[head(jt,h2), tt*128+t]
            pdq = pnorm.tile([P, 2, 2, 2], F32, tag="pn", name="pdq")
            for tt in range(2):
                for jt in range(2):
                    nc.tensor.matmul(
                        pdq[:, tt, jt, :],
                        eq_e[:, jt, tt * P:(tt + 1) * P],
                        eones1_sb,
                        start=True,
                        stop=True,
                    )
            rq4 = spool.tile([P, 2, 2, 2], F32, tag="rq4")
            nc.vector.reciprocal(rq4[:], pdq[:])

            # ---- v projection (natural layout) ----
            pv_t = ppool.tile([P, 2, J], F32, tag="pp")
            for st in range(2):
                n = 0
                vt = vterms(c)
                for whl, xhl in vt:
                    for kb in range(KB):
                        nc.tensor.matmul(
                            pv_t[:, st, :],
                            xk_t[:, xhl, kb, :, st * P:(st + 1) * P],
                            wa_sb[:, 2, whl, kb, :, :],
                            start=(n == 0),
                            stop=(n == len(vt) * KB - 1),
                            perf_mode=DR,
                        )
                        n += 1

            # ---- k denominators (64x, natural layout) -> v scale ----
            # pdk[s, jt, st, h2] = 64 * dk[head(jt,h2), st*128+s]
            pdk = pnorm.tile([P, 2, 2, 2], F32, tag="pn", name="pdk")
            for jt in range(2):
                for st in range(2):
                    nc.tensor.matmul(
                        pdk[:, jt, st, :],
                        ek_e[:, jt, st * P:(st + 1) * P],
                        eones64_sb,
                        start=True,
                        stop=True,
                    )
            rk = spool.tile([P, 2, 2, 2], F32, tag="rk")
            nc.vector.reciprocal(rk[:], pdk[:])
            v_sb = spool.tile([P, 2, J], BF16, tag="v_sb")
            for st in range(2):
                rk_b = rk[:, :, st, :, None].broadcast_to([P, 2, 2, DH])
                nc.vector.tensor_mul(
                    v_sb[:, st, :].rearrange("p (jt h2 d) -> p jt h2 d",
                                             jt=2, h2=2),
                    pv_t[:, st, :].rearrange("p (jt h2 d) -> p jt h2 d",
                                             jt=2, h2=2),
                    rk_b,
                )

            # ---- attention scores ----
            # pat[:, 0, :]      st=0, t 0:256   (diag block at cols 0:128)
            # pat[:, 1, 0:128]  st=1 diagonal block (t 128:256), shifted
            # masked bf16 tiles at_m[s, st, blk, 128]:
            #   [:, 0, 0] st0 diag, [:, 0, 1] st0 ones, [:, 1, 0] st1 diag
            at_mt = {}
            tri_b = tri_sb[:, None, :].broadcast_to([P, 2, P])
            for jt in range(2):
                for half in range(2):
                    rows = slice(64 * half, 64 * half + 64)
                    if last and jt == 1 and half == 1:
                        # state pool is idle in the last chunk; using its
                        # bank dodges the pat-slot WAR on the tail path
                        pat = pstat.tile([P, 2, C], F32, tag="ps",
                                         name="pat_last")
                    else:
                        pat = patp.tile([P, 2, C], F32, tag="pa",
                                        name=f"pat{jt}{half}")
                    nc.tensor.matmul(
                        pat[:, 0, :],
                        ek_e[rows, jt, 0:P],
                        eq_e[rows, jt, :],
                        start=True,
                        stop=True,
                    )
                    nc.tensor.matmul(
                        pat[:, 1, 0:P],
                        ek_e[rows, jt, P:2 * P],
                        eq_e[rows, jt, P:2 * P],
                        start=True,
                        stop=True,
                    )
                    at_m = apool.tile([P, 2, 2, P], BF16, tag="at",
                                      name=f"at{jt}{half}")
                    at_mt[(jt, half)] = at_m
                    # both triangular diag blocks in one strided multiply
                    nc.vector.tensor_mul(
                        at_m[:, :, 0, :], pat[:, :, 0:P], tri_b
                    )
                    # st0 "all attend" block: plain copy on ACT
                    nc.scalar.activation(
                        at_m[:, 0, 1, :], pat[:, 0, P:], COPY
                    )


            # ---- output (natural layout): out[t, j] = at^T v + eq^T S ----
            oc = spool.tile([P, 2, J], BF16, tag="oc")
            pot = potp.tile([P, 2, J], F32, tag="po")
            rq_b = rq4[:, :, :, :, None].broadcast_to([P, 2, 2, 2, DH])
            for tt in range(2):
                for jt in range(2):
                    for half in range(2):
                        h = 2 * jt + half
                        rows = slice(64 * half, 64 * half + 64)
                        jcols = slice(h * DH, (h + 1) * DH)
                        at_m = at_mt[(jt, half)]
                        # st=0 scores: diag block for tt0, ones block for tt1
                        nc.tensor.matmul(
                            pot[:, tt, jcols],
                            at_m[:, 0, tt, :],
                            v_sb[:, 0, jcols],
                            start=True,
                            stop=(c == 0 and tt == 0),
                        )
                        if tt == 1:
                            nc.tensor.matmul(
                                pot[:, tt, jcols],
                                at_m[:, 1, 0, :],
                                v_sb[:, 1, jcols],
                                start=False,
                                stop=(c == 0),
                            )
                        if c > 0:
                            nc.tensor.matmul(
                                pot[:, tt, jcols],
                                eq_e[rows, jt, tt * P:(tt + 1) * P],
                                S16[rows, jt, :],
                                start=False,
                                stop=True,
                            )
                nc.vector.tensor_mul(
                    oc[:, tt, :].rearrange("p (jt h2 d) -> p jt h2 d",
                                           jt=2, h2=2),
                    pot[:, tt, :].rearrange("p (jt h2 d) -> p jt h2 d",
                                            jt=2, h2=2),
                    rq_b[:, tt],
                )
                nc.sync.dma_start(out_r[:, c, tt], oc[:, tt, :])

            # ---- transpose ek to natural layout (for the state update) ----
            if not last:
                ekn_sb = spool.tile([P, 2, J], BF16, tag="ekn_sb")
                for jt in range(2):
                    ptr = pnorm.tile([P, 2, P], BF16, tag="pn",
                                     name=f"ptr{jt}")
                    for st in range(2):
                        nc.tensor.transpose(
                            ptr[:, st, :], ek_e[:, jt, st * P:(st + 1) * P],
                            ident_sb
                        )
                    nc.scalar.activation(
                        ekn_sb[:, :, jt * P:(jt + 1) * P], ptr[:], COPY
                    )
                state_update(c, ekn_sb, v_sb)




    nc.finalize()
    return nc


def _host_inputs(query, key, Wq, Wk, Wv):
    """Build the 8 per-core input maps (host-side layout prep)."""
    import ml_dtypes
    FP8NP = ml_dtypes.float8_e4m3
    bf = ml_dtypes.bfloat16

    def split_fp8(a):
        hi = a.astype(FP8NP)
        lo = (a - hi.astype(np.float32)).astype(FP8NP)
        return hi, lo

    def x_layout(xn, nhl):
        # xn [L, D] fp32 -> [P, NCH, nhl, KB, 2, C] fp8
        xt = np.ascontiguousarray(xn.T.astype(np.float32))  # [D, L]
        hi, lo = split_fp8(xt)
        out = np.empty((P, NCH, nhl, KB, 2, C), dtype=FP8NP)
        for i, a in enumerate((hi, lo)[:nhl]):
            # d = kb*256 + pl*128 + p ; t = c*256 + tt
            r = a.reshape(KB, 2, P, NCH, C)          # [kb, pl, p, c, t]
            out[:, :, i] = r.transpose(2, 3, 0, 1, 4)  # [p, c, kb, pl, t]
        return np.ascontiguousarray(out)

    def w_layout(Ws, cols):
        # -> [P, 3, 2, KB, 2, J] fp8, scaled by WSCALE
        out = np.zeros((P, 3, 2, KB, 2, J), dtype=FP8NP)
        for pi, W in enumerate(Ws):
            wt = np.ascontiguousarray(W[cols, :].T.astype(np.float32)) * WSCALE
            hi, lo = split_fp8(wt)                   # [D, J]
            for i, a in enumerate((hi, lo)):
                r = a.reshape(KB, 2, P, J)           # [kb, pl, p, j]
                out[:, pi, i] = r.transpose(2, 0, 1, 3)
        return np.ascontiguousarray(out)

    cst = np.zeros((P, 260), np.float32)
    cst[:, 0:P] = np.eye(P, dtype=np.float32)
    s = np.arange(P)[:, None]
    t = np.arange(P)[None, :]
    cst[:, P:2 * P] = (s <= t).astype(np.float32)
    cst[:64, 2 * P] = 1.0
    cst[64:, 2 * P + 1] = 1.0
    cst[:, 2 * P + 2:2 * P + 4] = cst[:, 2 * P:2 * P + 2] * WSCALE
    cst = cst.astype(bf)

    per_batch = {
        n: (x_layout(query[n], XQH), x_layout(key[n], XKH)) for n in range(2)
    }

    in_maps = []
    for core in range(N_CORES):
        n, g = core // 4, core % 4
        xq_a, xk_a = per_batch[n]
        cols = slice(g * J, (g + 1) * J)
        in_maps.append({
            "xq": xq_a,
            "xk": xk_a,
            "wa": w_layout((Wq, Wk, Wv), cols),
            "cst": cst,
        })
    return in_maps


_NC_LOCK = threading.Lock()
_NC_CACHE = {}


def _get_nc():
    with _NC_LOCK:
        if "nc" not in _NC_CACHE:
            _NC_CACHE["nc"] = _build_nc()
        return _NC_CACHE["nc"]


def kernel(query, key, Wq, Wk, Wv, _trace=False, _trace_kwargs=None):
    query = np.asarray(query)
    key = np.asarray(key)
    Wq = np.asarray(Wq)
    Wk = np.asarray(Wk)
    Wv = np.asarray(Wv)

    nc = _get_nc()
    in_maps = _host_inputs(query, key, Wq, Wk, Wv)
    res = run_bass_kernel_spmd(
        nc, in_maps, core_ids=list(range(N_CORES)),
        trace=_trace, **(_trace_kwargs or {}),
    )

    out = np.empty((2, L, D), np.float32)
    for core, r in enumerate(res.results):
        n, g = core // 4, core % 4
        out[n, :, g * J:(g + 1) * J] = np.asarray(r["out_nat"]).astype(np.float32)
    if _trace:
        kernel.last_results = res
    return out


# revision 47
# speedup vs baseline: 1.0191x; 1.0191x over previous
"""Trainium2 Bass kernel for causal linear-complexity multi-head attention.

Reference computation (per batch n):
    q = softmax((query @ Wq.T) per-head, axis=Dh)
    k = softmax((key  @ Wk.T) per-head, axis=Dh)
    v = key @ Wv.T
    S[t] = sum_{s<=t} k_s^T v_s          (per-head Dh x Dh running state)
    out[t] = q_t @ S[t]

Sharding: 8 cores = 2 batches x 4 head-groups (4 heads of 64 dims each per
core).  Each core gets host-transposed inputs (d on rows); per-core output
is the natural-layout (L x 256) slice; the host concatenates.

Device algorithm: chunked linear attention, chunk C=256.

Projections run as fp8e4m3 DoubleRow matmuls (2 contraction planes per
matmul, 0.5 cycles/row).  W ships as an fp8 hi/lo split (residual
re-rounded); x ships as fp8 hi (plus a lo residual plane for the early
v chunks).  The per-projection term count trades error for passes:
    q:         Wh@x8 + Wl@x8                       (2 passes, all chunks)
    k:         + Wl term only for chunks < K_LO_CHUNKS (else 1 pass)
    v:         + Wh@xlo term only for chunks < V_LO_CHUNKS
Early chunks keep more terms because their k/v errors persist in the
cumulative state S and touch every later output; late-chunk errors only
touch the few outputs after them.  Measured rel err 1.76e-2 vs the 2e-2
gate.  W is pre-scaled by 64 on host to center it in fp8 normal range;
undone by the exp activation scale (q/k paths) and a 64-valued
k-denominator ones vector (v path, scale rides the per-head reciprocal).

The attention block runs in bf16 and produces natural-layout output:
    scores   pat[s, t] = ek^T eq        (per head, T layout)
    output   out[t, j] = at^T v + eq^T S
Causal-structure zero blocks are never computed or touched:
  - score tile st=1 stores its diagonal block at columns 0:128, so both
    128x128 triangular blocks of a (head, t-tile) sit at a single strided
    AP and one DVE multiply masks both; the st=0 "all ones" block is a
    plain ACT copy (no mask needed).
  - the t-tile-0 output matmul skips the st=1 (all zero) score block.
The running state S accumulates IN PSUM across chunks (start only on the
very first update; later updates ride the accumulation bit), with a bf16
diagonal shadow extracted on ACT for the apply matmuls — no vector-engine
adds on the state path.  A PE warmup loop of identity transposes burns
the tensor-engine p-state ramp while the first input DMAs land; the
startup DMA stream is ordered by first use (the HWDGE descriptor
generator and the DMA wire are both serial devices).
"""

import threading
from contextlib import ExitStack

import numpy as np

import concourse.bass as bass
import concourse.mybir as mybir
import concourse.tile as tile
from concourse import bacc
from concourse.bass_utils import run_bass_kernel_spmd

P = 128          # SBUF partitions
D = 1024         # model dim (contraction)
J = 256          # per-core output columns (4 heads x 64)
L = 2048         # sequence length
C = 256          # chunk size
NCH = L // C     # chunks
DH = 64          # per-head dim
KB = 4           # 256-deep contraction blocks
N_CORES = 8
WSCALE = 64.0    # host pre-scale on W for fp8 range

# error/speed dials: how many leading chunks keep the extra fp8
# compensation terms (early-chunk k/v errors persist in the state S)
Q_LO = False      # q never ships an x residual
K_LO_CHUNKS = 3   # chunks with the 2-term (W-residual) k projection
V_LO = True
V_LO_CHUNKS = 5   # chunks with the 3-term v projection
WARMUP = 16       # identity transposes to burn the PE p-state ramp

F32 = mybir.dt.float32
BF16 = mybir.dt.bfloat16
FP8 = mybir.dt.float8e4
EXP = mybir.ActivationFunctionType.Exp
COPY = mybir.ActivationFunctionType.Copy
DR = mybir.MatmulPerfMode.DoubleRow

TERMS2 = ((0, 0), (1, 0))              # (w hi/lo, x plane)
TERMS1 = ((0, 0),)
TERMS2X = ((0, 0), (0, 1))             # W hi with x hi + x lo (k path)
TERMS3 = ((0, 0), (1, 0), (0, 1))      # + W_hi @ x_lo correction
XQH = 2 if Q_LO else 1
XKH = 2 if V_LO else 1


def _build_nc():
    nc = bacc.Bacc(trn_type="TRN2", target_bir_lowering=False, num_devices=N_CORES)

    # x: [p, chunk, hi(/lo), kb, plane, t]  with d = kb*256 + plane*128 + p
    xq = nc.dram_tensor("xq", [P, NCH, XQH, KB, 2, C], FP8, kind="ExternalInput").ap()
    xk = nc.dram_tensor("xk", [P, NCH, XKH, KB, 2, C], FP8, kind="ExternalInput").ap()
    # w: [p, proj(q,k,v), hi/lo, kb, plane, j]
    wa = nc.dram_tensor("wa", [P, 3, 2, KB, 2, J], FP8, kind="ExternalInput").ap()
    # packed consts: [0:128] ident, [128:256] tri mask, [256:258] ones,
    # [258:260] 64*ones
    cst = nc.dram_tensor("cst", [P, 260], BF16, kind="ExternalInput").ap()
    out_d = nc.dram_tensor("out_nat", [L, J], BF16, kind="ExternalOutput").ap()

    # natural-layout store: row = c*256 + tt*128 + p
    out_r = out_d.rearrange("(c tt p) j -> p c tt j", p=P, tt=2)

    with tile.TileContext(nc) as tc, ExitStack() as ctx:
        ctx.enter_context(
            nc.allow_low_precision(reason="fp8/bf16 compensated pipeline")
        )
        cpool = ctx.enter_context(tc.tile_pool(name="consts", bufs=1))
        xpool = ctx.enter_context(tc.tile_pool(name="xin", bufs=6))
        spool = ctx.enter_context(tc.tile_pool(name="sb", bufs=4))
        apool = ctx.enter_context(tc.tile_pool(name="at", bufs=8))
        ppool = ctx.enter_context(tc.tile_pool(name="pp", bufs=2, space="PSUM"))
        patp = ctx.enter_context(tc.tile_pool(name="pa", bufs=3, space="PSUM"))
        potp = ctx.enter_context(tc.tile_pool(name="po", bufs=1, space="PSUM"))
        pstat = ctx.enter_context(tc.tile_pool(name="ps", bufs=1, space="PSUM"))
        pnorm = ctx.enter_context(tc.tile_pool(name="pn", bufs=1, space="PSUM"))

        # ---- startup DMA stream (HWDGE is serial: needed-first order) ----
        cst_sb = cpool.tile([P, 260], BF16, tag="cst_sb")
        nc.sync.dma_start(cst_sb[:], cst[:])
        ident_sb = cst_sb[:, 0:P]
        tri_sb = cst_sb[:, P:2 * P]
        eones1_sb = cst_sb[:, 2 * P:2 * P + 2]
        eones64_sb = cst_sb[:, 2 * P + 2:2 * P + 4]

        # PE p-state warmup: transposes on the (tiny, first-landing) ident
        warm = pnorm.tile([P, P], BF16, tag="pn", name="warm")
        for _ in range(WARMUP):
            nc.tensor.transpose(warm[:], ident_sb, ident_sb)

        wa_sb = cpool.tile([P, 3, 2, KB, 2, J], FP8, tag="wa_sb")
        xq_t0 = xpool.tile([P, XQH, KB, 2, C], FP8, tag="xq_t", name="xq_t0")
        xk_t0 = xpool.tile([P, XKH, KB, 2, C], FP8, tag="xk_t", name="xk_t0")
        nc.sync.dma_start(wa_sb[:, 0, 0, 0:2], wa[:, 0, 0, 0:2])
        nc.sync.dma_start(xq_t0[:, 0, 0:2], xq[:, 0, 0, 0:2])
        nc.sync.dma_start(wa_sb[:, 0, 1], wa[:, 0, 1])
        nc.sync.dma_start(wa_sb[:, 0, 0, 2:4], wa[:, 0, 0, 2:4])
        nc.sync.dma_start(xq_t0[:, 0, 2:4], xq[:, 0, 0, 2:4])
        if Q_LO:
            nc.sync.dma_start(xq_t0[:, 1], xq[:, 0, 1])
        nc.sync.dma_start(wa_sb[:, 1, 0], wa[:, 1, 0])
        nc.sync.dma_start(xk_t0[:, 0], xk[:, 0, 0])
        nc.sync.dma_start(wa_sb[:, 1, 1], wa[:, 1, 1])
        if V_LO:
            nc.sync.dma_start(xk_t0[:, 1], xk[:, 0, 1])
        nc.sync.dma_start(wa_sb[:, 2, 0], wa[:, 2, 0])
        nc.sync.dma_start(wa_sb[:, 2, 1], wa[:, 2, 1])
        xq_t1 = xpool.tile([P, XQH, KB, 2, C], FP8, tag="xq_t", name="xq_t1")
        nc.sync.dma_start(xq_t1[:], xq[:, 1])
        xk_t1 = xpool.tile([P, XKH, KB, 2, C], FP8, tag="xk_t", name="xk_t1")
        nc.sync.dma_start(xk_t1[:, 0], xk[:, 1, 0])
        if V_LO:
            nc.sync.dma_start(xk_t1[:, 1], xk[:, 1, 1])

        S16 = cpool.tile([P, 2, DH], BF16, tag="S16")
        # resident PSUM accumulator for the running state (diag head blocks
        # of ekn^T v land at [64h:64h+64, jt, 64h:64h+64])
        pds = pstat.tile([P, 2, P], F32, tag="ps")

        qterms = TERMS3 if Q_LO else TERMS2

        def vterms(c):
            # early chunks feed every later output through S: keep the x
            # residual there; late chunks touch few outputs, drop it
            return (TERMS3 if c < V_LO_CHUNKS else TERMS2) if V_LO else TERMS2

        def dr_proj_T(pe_t, proj, x_t, terms):
            """q/k projection in transposed layout: out[j, t] per jt."""
            for jt in range(2):
                n = 0
                for whl, xhl in terms:
                    for kb in range(KB):
                        nc.tensor.matmul(
                            pe_t[:, jt, :],
                            wa_sb[:, proj, whl, kb, :, jt * P:(jt + 1) * P],
                            x_t[:, xhl, kb, :, :],
                            start=(n == 0),
                            stop=(n == len(terms) * KB - 1),
                            perf_mode=DR,
                        )
                        n += 1

        def state_update(c, ekn_sb, v_sb):
            # S += ekn^T v for chunk c, accumulated in the resident PSUM
            # bank (start only opens the zero region once, on the very
            # first matmul; later chunks accumulate in place)
            for jt in range(2):
                for st in range(2):
                    nc.tensor.matmul(
                        pds[:, jt, :],
                        ekn_sb[:, st, jt * P:(jt + 1) * P],
                        v_sb[:, st, jt * P:(jt + 1) * P],
                        start=(c == 0 and jt == 0 and st == 0),
                        stop=(st == 1),
                        skip_group_check=True,
                    )
            # bf16 diag shadow for the next chunk's apply matmuls; a full
            # chunk of slack, so deprioritize past the next chunk's exps
            with tc.high_priority(offset=-60):
                for half in range(2):
                    rows = slice(64 * half, 64 * half + 64)
                    cols = slice(64 * half, 64 * half + 64)
                    nc.scalar.activation(S16[rows, :, :], pds[rows, :, cols],
                                         COPY)

        xtiles = {0: (xq_t0, xk_t0), 1: (xq_t1, xk_t1)}
        for c in range(NCH):
            last = c == NCH - 1

            if c + 2 < NCH:
                xq_n = xpool.tile([P, XQH, KB, 2, C], FP8, tag="xq_t", name="xq_n")
                nc.sync.dma_start(xq_n[:], xq[:, c + 2])
                xk_n = xpool.tile([P, XKH, KB, 2, C], FP8, tag="xk_t", name="xk_n")
                if len(vterms(c + 2)) == 3:
                    nc.sync.dma_start(xk_n[:], xk[:, c + 2])
                else:
                    nc.sync.dma_start(xk_n[:, 0], xk[:, c + 2, 0])

                xtiles[c + 2] = (xq_n, xk_n)
            xq_t, xk_t = xtiles.pop(c)

            # ---- q/k projections (transposed layout) + exp ----
            eq_e = spool.tile([P, 2, C], BF16, tag="eq_e")
            ek_e = spool.tile([P, 2, C], BF16, tag="ek_e")
            pq_t = ppool.tile([P, 2, C], F32, tag="pp")
            dr_proj_T(pq_t, 0, xq_t, qterms)
            nc.scalar.activation(eq_e[:], pq_t[:], EXP, scale=1.0 / WSCALE)
            pk_t = ppool.tile([P, 2, C], F32, tag="pp")
            dr_proj_T(pk_t, 1, xk_t, TERMS2 if c < K_LO_CHUNKS else TERMS1)
            nc.scalar.activation(ek_e[:], pk_t[:], EXP, scale=1.0 / WSCALE)

            # ---- q denominators (natural layout, tiny) ----
            # pdq[t, tt, jt, h2] = dq[head(jt,h2), tt*128+t]
            pdq = pnorm.tile([P, 2, 2, 2], F32, tag="pn", name="pdq")
            for tt in range(2):
                for jt in range(2):
                    nc.tensor.matmul(
                        pdq[:, tt, jt, :],
                        eq_e[:, jt, tt * P:(tt + 1) * P],
                        eones1_sb,
                        start=True,
                        stop=True,
                    )
            rq4 = spool.tile([P, 2, 2, 2], F32, tag="rq4")
            nc.vector.reciprocal(rq4[:], pdq[:])

            # ---- v projection (natural layout) ----
            pv_t = ppool.tile([P, 2, J], F32, tag="pp")
            for st in range(2):
                n = 0
                vt = vterms(c)
                for whl, xhl in vt:
                    for kb in range(KB):
                        nc.tensor.matmul(
                            pv_t[:, st, :],
                            xk_t[:, xhl, kb, :, st * P:(st + 1) * P],
                            wa_sb[:, 2, whl, kb, :, :],
                            start=(n == 0),
                            stop=(n == len(vt) * KB - 1),
                            perf_mode=DR,
                        )
                        n += 1

            # ---- k denominators (64x, natural layout) -> v scale ----
            # pdk[s, jt, st, h2] = 64 * dk[head(jt,h2), st*128+s]
            pdk = pnorm.tile([P, 2, 2, 2], F32, tag="pn", name="pdk")
            for jt in range(2):
                for st in range(2):
                    nc.tensor.matmul(
                        pdk[:, jt, st, :],
                        ek_e[:, jt, st * P:(st + 1) * P],
                        eones64_sb,
                        start=True,
                        stop=True,
                    )
            rk = spool.tile([P, 2, 2, 2], F32, tag="rk")
            nc.vector.reciprocal(rk[:], pdk[:])
            v_sb = spool.tile([P, 2, J], BF16, tag="v_sb")
            for st in range(2):
                rk_b = rk[:, :, st, :, None].broadcast_to([P, 2, 2, DH])
                nc.vector.tensor_mul(
                    v_sb[:, st, :].rearrange("p (jt h2 d) -> p jt h2 d",
                                             jt=2, h2=2),
                    pv_t[:, st, :].rearrange("p (jt h2 d) -> p jt h2 d",
                                             jt=2, h2=2),
                    rk_b,
                )

            # ---- attention scores ----
            # pat[:, 0, :]      st=0, t 0:256   (diag block at cols 0:128)
            # pat[:, 1, 0:128]  st=1 diagonal block (t 128:256), shifted
            # masked bf16 tiles at_m[s, st, blk, 128]:
            #   [:, 0, 0] st0 diag, [:, 0, 1] st0 ones, [:, 1, 0] st1 diag
            at_mt = {}
            tri_b = tri_sb[:, None, :].broadcast_to([P, 2, P])
            for jt in range(2):
                for half in range(2):
                    rows = slice(64 * half, 64 * half + 64)
                    if last and jt == 1 and half == 1:
                        # state pool is idle in the last chunk; using its
                        # bank dodges the pat-slot WAR on the tail path
                        pat = pstat.tile([P, 2, C], F32, tag="ps",
                                         name="pat_last")
                    else:
                        pat = patp.tile([P, 2, C], F32, tag="pa",
                                        name=f"pat{jt}{half}")
                    nc.tensor.matmul(
                        pat[:, 0, :],
                        ek_e[rows, jt, 0:P],
                        eq_e[rows, jt, :],
                        start=True,
                        stop=True,
                    )
                    nc.tensor.matmul(
                        pat[:, 1, 0:P],
                        ek_e[rows, jt, P:2 * P],
                        eq_e[rows, jt, P:2 * P],
                        start=True,
                        stop=True,
                    )
                    at_m = apool.tile([P, 2, 2, P], BF16, tag="at",
                                      name=f"at{jt}{half}")
                    at_mt[(jt, half)] = at_m
                    # both triangular diag blocks in one strided multiply
                    nc.vector.tensor_mul(
                        at_m[:, :, 0, :], pat[:, :, 0:P], tri_b
                    )
                    # st0 "all attend" block: plain copy on ACT
                    nc.scalar.activation(
                        at_m[:, 0, 1, :], pat[:, 0, P:], COPY
                    )


            # ---- output (natural layout): out[t, j] = at^T v + eq^T S ----
            oc = spool.tile([P, 2, J], BF16, tag="oc")
            pot = potp.tile([P, 2, J], F32, tag="po")
            rq_b = rq4[:, :, :, :, None].broadcast_to([P, 2, 2, 2, DH])
            for tt in range(2):
                for jt in range(2):
                    for half in range(2):
                        h = 2 * jt + half
                        rows = slice(64 * half, 64 * half + 64)
                        jcols = slice(h * DH, (h + 1) * DH)
                        at_m = at_mt[(jt, half)]
                        # st=0 scores: diag block for tt0, ones block for tt1
                        nc.tensor.matmul(
                            pot[:, tt, jcols],
                            at_m[:, 0, tt, :],
                            v_sb[:, 0, jcols],
                            start=True,
                            stop=(c == 0 and tt == 0),
                        )
                        if tt == 1:
                            nc.tensor.matmul(
                                pot[:, tt, jcols],
                                at_m[:, 1, 0, :],
                                v_sb[:, 1, jcols],
                                start=False,
                                stop=(c == 0),
                            )
                        if c > 0:
                            nc.tensor.matmul(
                                pot[:, tt, jcols],
                                eq_e[rows, jt, tt * P:(tt + 1) * P],
                                S16[rows, jt, :],
                                start=False,
                                stop=True,
                            )
                nc.vector.tensor_mul(
                    oc[:, tt, :].rearrange("p (jt h2 d) -> p jt h2 d",
                                           jt=2, h2=2),
                    pot[:, tt, :].rearrange("p (jt h2 d) -> p jt h2 d",
                                            jt=2, h2=2),
                    rq_b[:, tt],
                )
                nc.sync.dma_start(out_r[:, c, tt], oc[:, tt, :])

            # ---- transpose ek to natural layout (for the state update) ----
            if not last:
                ekn_sb = spool.tile([P, 2, J], BF16, tag="ekn_sb")
                for jt in range(2):
                    ptr = pnorm.tile([P, 2, P], BF16, tag="pn",
                                     name=f"ptr{jt}")
                    for st in range(2):
                        nc.tensor.transpose(
                            ptr[:, st, :], ek_e[:, jt, st * P:(st + 1) * P],
                            ident_sb
                        )
                    nc.scalar.activation(
                        ekn_sb[:, :, jt * P:(jt + 1) * P], ptr[:], COPY
                    )
                state_update(c, ekn_sb, v_sb)




    nc.finalize()
    return nc


def _host_inputs(query, key, Wq, Wk, Wv):
    """Build the 8 per-core input maps (host-side layout prep)."""
    import ml_dtypes
    FP8NP = ml_dtypes.float8_e4m3
    bf = ml_dtypes.bfloat16

    def split_fp8(a):
        hi = a.astype(FP8NP)
        lo = (a - hi.astype(np.float32)).astype(FP8NP)
        return hi, lo

    def x_layout(xn, nhl):
        # xn [L, D] fp32 -> [P, NCH, nhl, KB, 2, C] fp8
        xt = np.ascontiguousarray(xn.T.astype(np.float32))  # [D, L]
        hi, lo = split_fp8(xt)
        out = np.empty((P, NCH, nhl, KB, 2, C), dtype=FP8NP)
        for i, a in enumerate((hi, lo)[:nhl]):
            # d = kb*256 + pl*128 + p ; t = c*256 + tt
            r = a.reshape(KB, 2, P, NCH, C)          # [kb, pl, p, c, t]
            out[:, :, i] = r.transpose(2, 3, 0, 1, 4)  # [p, c, kb, pl, t]
        return np.ascontiguousarray(out)

    def w_layout(Ws, cols):
        # -> [P, 3, 2, KB, 2, J] fp8, scaled by WSCALE
        out = np.zeros((P, 3, 2, KB, 2, J), dtype=FP8NP)
        for pi, W in enumerate(Ws):
            wt = np.ascontiguousarray(W[cols, :].T.astype(np.float32)) * WSCALE
            hi, lo = split_fp8(wt)                   # [D, J]
            for i, a in enumerate((hi, lo)):
                r = a.reshape(KB, 2, P, J)           # [kb, pl, p, j]
                out[:, pi, i] = r.transpose(2, 0, 1, 3)
        return np.ascontiguousarray(out)

    cst = np.zeros((P, 260), np.float32)
    cst[:, 0:P] = np.eye(P, dtype=np.float32)
    s = np.arange(P)[:, None]
    t = np.arange(P)[None, :]
    cst[:, P:2 * P] = (s <= t).astype(np.float32)
    cst[:64, 2 * P] = 1.0
    cst[64:, 2 * P + 1] = 1.0
    cst[:, 2 * P + 2:2 * P + 4] = cst[:, 2 * P:2 * P + 2] * WSCALE
    cst = cst.astype(bf)

    per_batch = {
        n: (x_layout(query[n], XQH), x_layout(key[n], XKH)) for n in range(2)
    }

    in_maps = []
    for core in range(N_CORES):
        n, g = core // 4, core % 4
        xq_a, xk_a = per_batch[n]
        cols = slice(g * J, (g + 1) * J)
        in_maps.append({
            "xq": xq_a,
            "xk": xk_a,
            "wa": w_layout((Wq, Wk, Wv), cols),
            "cst": cst,
        })
    return in_maps


_NC_LOCK = threading.Lock()
_NC_CACHE = {}


def _get_nc():
    with _NC_LOCK:
        if "nc" not in _NC_CACHE:
            _NC_CACHE["nc"] = _build_nc()
        return _NC_CACHE["nc"]


def kernel(query, key, Wq, Wk, Wv, _trace=False, _trace_kwargs=None):
    query = np.asarray(query)
    key = np.asarray(key)
    Wq = np.asarray(Wq)
    Wk = np.asarray(Wk)
    Wv = np.asarray(Wv)

    nc = _get_nc()
    in_maps = _host_inputs(query, key, Wq, Wk, Wv)
    res = run_bass_kernel_spmd(
        nc, in_maps, core_ids=list(range(N_CORES)),
        trace=_trace, **(_trace_kwargs or {}),
    )

    out = np.empty((2, L, D), np.float32)
    for core, r in enumerate(res.results):
        n, g = core // 4, core % 4
        out[n, :, g * J:(g + 1) * J] = np.asarray(r["out_nat"]).astype(np.float32)
    if _trace:
        kernel.last_results = res
    return out
